# revision 11
# baseline (speedup 1.0000x reference)
import sys
import os

sys.path.insert(0, "/opt/trn_rl_repo")

import numpy as np
import ml_dtypes

import concourse.bass as bass
import concourse.mybir as mybir
import concourse.tile as tile
from concourse import bacc
from concourse.bass_utils import run_bass_kernel_spmd

BF16 = ml_dtypes.bfloat16

# model dims (fixed by the problem)
SITE_PROPS = 92
SITE_EMB = 64
BOND_EMB = 64
BOND_EXP = 64
MAX_DIST = 8.0
H1, H2, OUT = 128, 64, 1
N_GRAPHS = 512
GCHUNKS = N_GRAPHS // 128

F32 = mybir.dt.float32
BF = mybir.dt.bfloat16
I16 = mybir.dt.int16

STEP = MAX_DIST / BOND_EXP

# Gaussian-basis lookup table resolution (d quantized to MAX_DIST/NLEV)
NLEV = 4096
QSCALE = NLEV / MAX_DIST
GRANK = NLEV // 128


class Cfg:
    def __init__(self, n_cores, nblk, T, n_sites, n_graphs=N_GRAPHS,
                 gate_bias=False):
        self.n_cores = n_cores
        self.nblk = nblk          # 128-site blocks per core (must be even)
        self.T = T                # edge tiles (128 edges) per block, = 4*Tq
        self.R = nblk * 128       # sites per core
        self.site_pad = n_cores * self.R
        self.n_sites = n_sites
        self.n_graphs = n_graphs
        self.gate_bias = gate_bias
        self.Tq = None
        # pair-token split so idx fits int16: low = pairs [0, SPLIT),
        # high = pairs [SPLIT, npair)
        self.npair = self.site_pad // 2
        self.split = self.npair // 2
        assert self.split <= 32768 and self.npair - self.split <= 32768
        self.sbuf_table = True    # gather site table from SBUF (else HBM)
        self.phases = "E12PH"


def _plview(slice_ap, b, nsb):
    """Pair-layout DRAM slice view for block b as a [128, 64] AP.

    Token l of a core pairs local sites (l, l + R/2); DRAM layout is
    token-major [R/2, 128]. Block b < nsb is the first half (cols 0:64)
    of token rows [b*128, (b+1)*128); block b >= nsb the second half.
    """
    h = 64 * (b // nsb)
    r = (b % nsb) * 128
    return slice_ap[r:r + 128, h:h + 64]


def build_graph_kernel(nc, tc, ins, outs, cfg):
    NBLK, T, Tq, R = cfg.nblk, cfg.T, cfg.Tq, cfg.R
    NC = cfg.n_cores
    NSB = NBLK // 2
    NRANK = cfg.npair // 128          # pair ranks in the full table
    LO_RANKS = cfg.split // 128       # low-half view: pairs [0, split)
    lo_elems = LO_RANKS * 128         # bf16 elems per partition in low view
    hi_elems = (NRANK - LO_RANKS) * 128

    groups = []
    t0 = 0
    while t0 < T:
        g = min(8, T - t0)
        groups.append((t0, g))
        t0 += g

    from contextlib import ExitStack
    stack = ExitStack()
    dram = stack.enter_context(tc.tile_pool(name="dram", bufs=1, space="DRAM"))
    # pair-layout slices + gathered tables
    Rh = R // 2
    embS = dram.tile([Rh, 128], BF)
    l1S = dram.tile([Rh, 128], BF)
    l2S = dram.tile([Rh, 128], BF)
    tabA = dram.tile([NC * Rh, 128], BF, addr_space="Shared")
    tabB = dram.tile([NC * Rh, 128], BF, addr_space="Shared")
    pool_part = dram.tile([cfg.n_graphs, SITE_EMB + 1], F32)
    pool_full = dram.tile([cfg.n_graphs, SITE_EMB + 1], F32,
                          addr_space="Shared")

    cp = stack.enter_context(tc.tile_pool(name="consts", bufs=1))

    def load_const(name, shape, dtype):
        t = cp.tile(shape, dtype, name=f"c_{name}", tag=f"c_{name}")
        nc.sync.dma_start(t[:], ins[name][:])
        return t

    identity = load_const("identity128", [128, 128], F32)
    identity_bf = load_const("identity128_bf", [128, 128], BF)
    iota512 = load_const("iota512", [128, cfg.n_graphs], F32)
    iota_grp = load_const("iota_grp", [128, 8 * 128], BF)
    ones_bf = load_const("ones_col", [128, 1], BF)
    Wse = load_const("Wse", [SITE_PROPS, SITE_EMB], F32)
    bse = load_const("bse", [SITE_EMB, 1], F32)
    Wcat1 = {}
    Ws2dup = {}
    Wbb = {}
    for L in (1, 2):
        Wcat1[L] = load_const(f"W{L}_cat1", [SITE_EMB, 2 * SITE_EMB], BF)
        Ws2dup[(L, 0)] = load_const(f"W{L}_s2v0", [128, 2 * SITE_EMB], BF)
        Ws2dup[(L, 1)] = load_const(f"W{L}_s2v1", [128, 2 * SITE_EMB], BF)
        Wbb[L] = load_const(f"W{L}_bb", [SITE_EMB, 2 * SITE_EMB], BF)
    bgate = {}
    if cfg.gate_bias:
        for L in (1, 2):
            bgate[L] = load_const(f"b{L}_row2", [128, 8 * 128], F32)
    W1 = load_const("W1", [SITE_EMB, H1], F32)
    b1 = load_const("b1", [H1, 1], F32)
    W2 = load_const("W2", [H1, H2], F32)
    b2 = load_const("b2", [H2, 1], F32)
    W3 = load_const("W3", [H2, OUT], F32)
    b3 = load_const("b3", [1, 1], F32)

    # persistent SBUF: site table (pair tokens) + gaussian LUT
    if cfg.sbuf_table:
        tp = stack.enter_context(tc.tile_pool(name="table", bufs=1))
        tab_sb = tp.tile([128, NRANK * 128], BF, name="tab_sb")
        G_sb = tp.tile([128, GRANK * 128], BF, name="G_sb")
        nc.sync.dma_start(G_sb[:], ins["Gtab"][:])
    else:
        Gdram = dram.tile([NLEV, 128], BF)
        nc.sync.dma_start(
            Gdram[:].rearrange("(r p) e -> p r e", p=128),
            ins["Gtab"][:].rearrange("p (r e) -> p r e", e=128))

    # ---------------- Phase E: site embedding (own slice) ----------------
    if "E" in cfg.phases:
        with (
            tc.tile_pool(name="emb_sb", bufs=3) as esb,
            tc.tile_pool(name="emb_ps", bufs=2, space="PSUM") as eps,
        ):
            for b in range(NBLK):
                srow = esb.tile([128, SITE_PROPS], F32, tag="srow")
                nc.sync.dma_start(srow[:],
                                  ins["sites_slice"][b * 128:(b + 1) * 128, :])
                stp = eps.tile([SITE_PROPS, 128], F32, tag="stp", space="PSUM")
                nc.tensor.transpose(stp[:], srow[:], identity[:])
                sts = esb.tile([SITE_PROPS, 128], F32, tag="sts")
                nc.vector.tensor_copy(sts[:], stp[:])
                s0T = eps.tile([SITE_EMB, 128], F32, tag="s0T", space="PSUM")
                nc.tensor.matmul(s0T[:], lhsT=Wse[:], rhs=sts[:],
                                 start=True, stop=True)
                s0Tb = esb.tile([SITE_EMB, 128], F32, tag="s0Tb")
                nc.vector.tensor_scalar_add(s0Tb[:], s0T[:], bse[:, 0:1])
                s0p = eps.tile([128, SITE_EMB], F32, tag="s0p", space="PSUM")
                nc.tensor.transpose(s0p[:], s0Tb[:],
                                    identity[:SITE_EMB, :SITE_EMB])
                s0row = esb.tile([128, SITE_EMB], BF, tag="s0row")
                nc.vector.tensor_copy(s0row[:], s0p[:])
                nc.sync.dma_start(_plview(embS, b, NSB), s0row[:])

        nc.gpsimd.collective_compute(
            "AllGather", mybir.AluOpType.bypass,
            replica_groups=[list(range(NC))],
            ins=[embS.opt()], outs=[tabA.opt()],
        )

    # ---------------- conv layers ----------------
    def load_table(tab_pl):
        # DRAM token-major [NTOK, 128] -> SBUF [128 part, rank*128]
        if cfg.sbuf_table:
            nc.sync.dma_start(
                tab_sb[:].rearrange("p (r e) -> p r e", e=128),
                tab_pl[:].rearrange("(r p) e -> p r e", p=128))

    def conv_layer(L, slice_in, slice_out, tab_dram):
        with (
            tc.tile_pool(name=f"c{L}_io", bufs=2) as iop,
            tc.tile_pool(name=f"c{L}_gt", bufs=2) as gtp,
            tc.tile_pool(name=f"c{L}_ms", bufs=2) as msb,
            tc.tile_pool(name=f"c{L}_ps", bufs=2, space="PSUM") as cps,
            tc.tile_pool(name=f"c{L}_aps", bufs=2, space="PSUM") as aps,
            tc.tile_pool(name=f"c{L}_tps", bufs=1, space="PSUM") as tps,
        ):
            for sbi in range(NSB):
                bA = 2 * sbi
                # merged low/high pair gathers over both blocks
                i2lo = iop.tile([128, 4 * Tq * 8], I16, tag="i2lo")
                nc.sync.dma_start(i2lo[:], ins["i2lo_blk"][sbi, :, :])
                i2hi = iop.tile([128, 4 * Tq * 8], I16, tag="i2hi")
                nc.sync.dma_start(i2hi[:], ins["i2hi_blk"][sbi, :, :])
                g2lo = gtp.tile([128, 1, 4 * Tq * 128], BF, tag="g2lo")
                g2hi = gtp.tile([128, 1, 4 * Tq * 128], BF, tag="g2hi")
                if cfg.sbuf_table:
                    nc.gpsimd.dma_gather(
                        g2lo[:], tab_sb[:, 0:lo_elems], i2lo[:],
                        4 * Tq * 128, 4 * Tq * 128, 128, transpose=True,
                        sbuf_tokens_per_rank=128, sbuf_free_dim_per_rank=256,
                        single_packet=False)
                    nc.gpsimd.dma_gather(
                        g2hi[:], tab_sb[:, lo_elems:lo_elems + hi_elems],
                        i2hi[:],
                        4 * Tq * 128, 4 * Tq * 128, 128, transpose=True,
                        sbuf_tokens_per_rank=128, sbuf_free_dim_per_rank=256,
                        single_packet=False)
                else:
                    nc.gpsimd.dma_gather(
                        g2lo[:], tab_dram[0:cfg.split, :], i2lo[:],
                        4 * Tq * 128, 4 * Tq * 128, 128, transpose=True,
                        single_packet=False)
                    nc.gpsimd.dma_gather(
                        g2hi[:], tab_dram[cfg.split:cfg.npair, :], i2hi[:],
                        4 * Tq * 128, 4 * Tq * 128, 128, transpose=True,
                        single_packet=False)
                # per-superblock bond-expansion LUT gather (both blocks)
                qlut = iop.tile([128, 2 * T * 8], I16, tag="qlut")
                nc.sync.dma_start(qlut[:], ins["qlut_blk"][sbi, :, :])
                bex = gtp.tile([128, 1, 2 * T * 128], BF, tag="bex")
                if cfg.sbuf_table:
                    nc.gpsimd.dma_gather(
                        bex[:], G_sb[:], qlut[:],
                        2 * T * 128, 2 * T * 128, 128, transpose=True,
                        sbuf_tokens_per_rank=128, sbuf_free_dim_per_rank=256,
                        single_packet=False)
                else:
                    nc.gpsimd.dma_gather(
                        bex[:], Gdram[:], qlut[:],
                        2 * T * 128, 2 * T * 128, 128, transpose=True,
                        single_packet=False)
                s8t = iop.tile([128, 2, T * 128], BF, tag="s8t")
                nc.sync.dma_start(s8t[:], ins["s8t_blk"][sbi, :, :, :])
                rel = iop.tile([128, 2, T], BF, tag="rel")
                nc.sync.dma_start(rel[:], ins["rel_blk"][sbi, :, :, :])
                sblk2 = iop.tile([128, 2, SITE_EMB], BF, tag="sblk2")
                nc.sync.dma_start(
                    sblk2[:, 0, :], _plview(slice_in, bA, NSB))
                nc.sync.dma_start(
                    sblk2[:, 1, :], _plview(slice_in, bA + 1, NSB))

                for half in (0, 1):
                    b = bA + half
                    # P_blk = sblk @ [Wsig_s1 | Wsof_s1]
                    sbT_ps = tps.tile([SITE_EMB, 128], BF, tag="sbT_ps",
                                      space="PSUM")
                    nc.tensor.transpose(sbT_ps[:], sblk2[:, half, :],
                                        identity_bf[:])
                    sbT = msb.tile([SITE_EMB, 128], BF, tag="sbT")
                    nc.vector.tensor_copy(sbT[:], sbT_ps[:])
                    pblk_ps = tps.tile([128, 128], F32, tag="pblk_ps",
                                       space="PSUM")
                    nc.tensor.matmul(pblk_ps[:], lhsT=sbT[:], rhs=Wcat1[L][:],
                                     start=True, stop=True)
                    pblk = msb.tile([128, 128], BF, tag="pblk")
                    nc.vector.tensor_copy(pblk[:], pblk_ps[:])

                    aggP = aps.tile([128, SITE_EMB], F32, tag="aggP",
                                    space="PSUM")
                    for (gs, gl) in groups:
                        gps2 = cps.tile([128, 8, 128], F32, tag="gps2",
                                        space="PSUM")
                        for ti in range(gl):
                            t = gs + ti
                            par = (t // Tq) & 1
                            if t < 2 * Tq:
                                src2 = g2lo[:, 0,
                                            (half * 2 * Tq + t) * 128:
                                            (half * 2 * Tq + t + 1) * 128]
                            else:
                                th = t - 2 * Tq
                                src2 = g2hi[:, 0,
                                            (half * 2 * Tq + th) * 128:
                                            (half * 2 * Tq + th + 1) * 128]
                            nc.tensor.matmul(
                                gps2[:, ti, :],
                                lhsT=s8t[:, half, t * 128:(t + 1) * 128],
                                rhs=pblk[:], start=True, stop=False)
                            nc.tensor.matmul(
                                gps2[:, ti, :], lhsT=src2,
                                rhs=Ws2dup[(L, par)][:],
                                start=False, stop=False)
                            nc.tensor.matmul(
                                gps2[:, ti, :],
                                lhsT=bex[0:64, 0,
                                         (half * T + t) * 128:
                                         (half * T + t + 1) * 128],
                                rhs=Wbb[L][:], start=False, stop=True)
                        if cfg.gate_bias:
                            nc.vector.tensor_tensor(
                                out=gps2[:, 0:gl, :],
                                in0=gps2[:, 0:gl, :],
                                in1=bgate[L][:].rearrange(
                                    "p (a b) -> p a b", b=128)[:, 0:gl, :],
                                op=mybir.AluOpType.add)
                        asig = msb.tile([128, 8, SITE_EMB], BF, tag="asig")
                        nc.scalar.activation(
                            asig[:, 0:gl, :], gps2[:, 0:gl, 0:SITE_EMB],
                            mybir.ActivationFunctionType.Tanh, scale=0.5)
                        asof = msb.tile([128, 8, SITE_EMB], BF, tag="asof")
                        nc.scalar.activation(
                            asof[:, 0:gl, :], gps2[:, 0:gl, SITE_EMB:128],
                            mybir.ActivationFunctionType.Relu, scale=0.5)
                        # gmsg = (tanh+1) * 0.5relu = sigmoid * relu
                        gmsg = msb.tile([128, 8, SITE_EMB], BF, tag="gmsg")
                        nc.vector.scalar_tensor_tensor(
                            gmsg[:, 0:gl, :], asig[:, 0:gl, :], 1.0,
                            asof[:, 0:gl, :],
                            op0=mybir.AluOpType.add, op1=mybir.AluOpType.mult)
                        S8 = msb.tile([128, 8, 128], BF, tag="S8")
                        nc.vector.tensor_tensor(
                            out=S8[:, 0:gl, :],
                            in0=iota_grp[:].rearrange(
                                "p (a b) -> p a b", b=128)[:, 0:gl, :],
                            in1=rel[:, half, gs:gs + gl].to_broadcast(
                                [128, gl, 128]),
                            op=mybir.AluOpType.is_equal)
                        for ti in range(gl):
                            t = gs + ti
                            nc.tensor.matmul(
                                aggP[:], lhsT=S8[:, ti, :],
                                rhs=gmsg[:, ti, :],
                                start=(t == 0), stop=(t == T - 1),
                                skip_group_check=True)
                    snew = msb.tile([128, SITE_EMB], BF, tag="snew")
                    nc.vector.tensor_tensor(
                        out=snew[:], in0=aggP[:], in1=sblk2[:, half, :],
                        op=mybir.AluOpType.add)
                    nc.sync.dma_start(_plview(slice_out, b, NSB), snew[:])

    if "1" in cfg.phases:
        load_table(tabA)
        conv_layer(1, embS, l1S, tabA)
        nc.gpsimd.collective_compute(
            "AllGather", mybir.AluOpType.bypass,
            replica_groups=[list(range(NC))],
            ins=[l1S.opt()], outs=[tabB.opt()],
        )
    if "2" in cfg.phases:
        load_table(tabB)
        conv_layer(2, l1S, l2S, tabB)

    # ---------------- Phase P: pooling over own sites ----------------
    if "P" in cfg.phases:
        with (
            tc.tile_pool(name="pool_sb", bufs=3) as psb,
            tc.tile_pool(name="pool_ps", bufs=1, space="PSUM") as pps,
        ):
            pool_ps = [
                pps.tile([128, SITE_EMB + 1], F32, tag=f"pool{c}",
                         space="PSUM", name=f"pool_ps{c}")
                for c in range(GCHUNKS)
            ]
            for b in range(NBLK):
                rhs = psb.tile([128, SITE_EMB + 1], BF, tag="prhs")
                nc.sync.dma_start(rhs[:, 0:SITE_EMB], _plview(l2S, b, NBLK // 2))
                nc.vector.tensor_copy(rhs[:, SITE_EMB:SITE_EMB + 1],
                                      ones_bf[:])
                gid = psb.tile([128, 1], F32, tag="gid")
                nc.sync.dma_start(gid[:], ins["gid_blk"][b, :, None])
                Sp = psb.tile([128, cfg.n_graphs], BF, tag="Spool")
                nc.vector.tensor_tensor(
                    out=Sp[:],
                    in0=gid[:, 0:1].to_broadcast([128, cfg.n_graphs]),
                    in1=iota512[:], op=mybir.AluOpType.is_equal)
                for c in range(GCHUNKS):
                    nc.tensor.matmul(
                        pool_ps[c][:], lhsT=Sp[:, c * 128:(c + 1) * 128],
                        rhs=rhs[:], start=(b == 0), stop=(b == NBLK - 1),
                        skip_group_check=True)
            pstage = psb.tile([128, GCHUNKS, SITE_EMB + 1], F32, tag="pstage")
            for c in range(GCHUNKS):
                nc.vector.tensor_copy(pstage[:, c, :], pool_ps[c][:])
            nc.sync.dma_start(
                pool_part[:].rearrange("(c p) f -> p c f", p=128), pstage[:])

        nc.gpsimd.collective_compute(
            "AllReduce", mybir.AluOpType.add,
            replica_groups=[list(range(NC))],
            ins=[pool_part.opt()], outs=[pool_full.opt()],
        )

    # ---------------- Phase H: head MLP (replicated) ----------------
    if "H" in cfg.phases:
        with (
            tc.tile_pool(name="head_sb", bufs=1) as hsb,
            tc.tile_pool(name="head_ps", bufs=1, space="PSUM") as hps,
        ):
            pool_sb = hsb.tile([128, GCHUNKS, SITE_EMB + 1], F32)
            nc.sync.dma_start(
                pool_sb[:], pool_full[:].rearrange("(c p) f -> p c f", p=128))
            vecT = hsb.tile([SITE_EMB, GCHUNKS * 128], F32)
            for c in range(GCHUNKS):
                cnt = hsb.tile([128, 1], F32, tag="cnt")
                nc.vector.tensor_scalar_max(cnt[:], pool_sb[:, c, SITE_EMB:],
                                            1.0)
                rec = hsb.tile([128, 1], F32, tag="rec")
                nc.vector.reciprocal(rec[:], cnt[:])
                vc = hsb.tile([128, SITE_EMB], F32, tag="vc")
                nc.vector.tensor_scalar_mul(vc[:], pool_sb[:, c, 0:SITE_EMB],
                                            rec[:, 0:1])
                vtp = hps.tile([SITE_EMB, 128], F32, tag="vtp", space="PSUM")
                nc.tensor.transpose(vtp[:], vc[:], identity[:])
                nc.vector.tensor_copy(vecT[:, c * 128:(c + 1) * 128], vtp[:])
            h1p = hps.tile([H1, cfg.n_graphs], F32, tag="h1p", space="PSUM")
            nc.tensor.matmul(h1p[:], lhsT=W1[:], rhs=vecT[:], start=True,
                             stop=True)
            h1 = hsb.tile([H1, cfg.n_graphs], F32)
            nc.scalar.activation(h1[:], h1p[:],
                                 mybir.ActivationFunctionType.Relu,
                                 bias=b1[:, 0:1])
            h2p = hps.tile([H2, cfg.n_graphs], F32, tag="h2p", space="PSUM")
            nc.tensor.matmul(h2p[:], lhsT=W2[:], rhs=h1[:], start=True,
                             stop=True)
            h2 = hsb.tile([H2, cfg.n_graphs], F32)
            nc.scalar.activation(h2[:], h2p[:],
                                 mybir.ActivationFunctionType.Relu,
                                 bias=b2[:, 0:1])
            op = hps.tile([OUT, cfg.n_graphs], F32, tag="op", space="PSUM")
            nc.tensor.matmul(op[:], lhsT=W3[:], rhs=h2[:], start=True,
                             stop=True)
            ot = hsb.tile([OUT, cfg.n_graphs], F32)
            nc.vector.tensor_scalar_add(ot[:], op[:], b3[:, 0:1])
            nc.sync.dma_start(outs["out"][:].rearrange("g o -> o g"), ot[:])

    stack.close()


# ======================================================================
# Host-side preparation (pure data movement / index planning)
# ======================================================================

def _wrap16(idx):
    """[..., n] int -> [..., 128, n//16] int16 gather index layout."""
    *lead, n = idx.shape
    w = idx.reshape(*lead, n // 16, 16)
    w = np.moveaxis(w, -1, -2)          # [..., 16, n//16]
    w = np.concatenate([w] * 8, axis=-2)  # replicate to 128 partitions
    return np.ascontiguousarray(w.astype(np.int16))


def prep_host(inputs, cfg):
    NC, NBLK, R = cfg.n_cores, cfg.nblk, cfg.R
    T0 = cfg.T
    i1 = np.asarray(inputs["indices1"]).astype(np.int64)
    i2 = np.asarray(inputs["indices2"]).astype(np.int64)
    bonds = np.asarray(inputs["bonds"], dtype=np.float32)
    n_sites = cfg.n_sites

    blk = i1 >> 7
    nblk_tot = NC * NBLK
    Rh = R // 2
    c2 = i2 // R
    ll = i2 % R
    tok_all = c2 * Rh + (ll % Rh)
    half_all = ll // Rh
    cls = (tok_all >= cfg.split).astype(np.int64) * 2 + half_all
    key = blk * 4 + cls
    order = np.argsort(key, kind="stable")
    i1s, bs, keys = i1[order], bonds[order], key[order]
    tok_s = tok_all[order]
    blks = blk[order]

    cnts = np.bincount(keys, minlength=nblk_tot * 4)
    Tq = max(1, int(np.ceil(cnts.max() / 128.0)))
    T = 4 * Tq
    if T0 is not None:
        assert T <= T0, f"data needs T={T} > configured {T0}"
        T = T0
        Tq = T // 4
    cfg.T = T
    cfg.Tq = Tq

    cap = T * 128
    starts = np.zeros(nblk_tot * 4 + 1, dtype=np.int64)
    np.cumsum(cnts, out=starts[1:])
    within = np.arange(len(i1s), dtype=np.int64) - starts[keys]
    slots = blks * cap + (keys % 4) * (Tq * 128) + within

    def scatter_flat(vals, fill, dtype):
        out = np.full(nblk_tot * cap, fill, dtype=dtype)
        out[slots] = vals.astype(dtype)
        return out.reshape(NC, NBLK, T, 128)

    # i2 pair tokens, low/high split
    tok = np.where(tok_s < cfg.split, tok_s, tok_s - cfg.split)
    tokg = scatter_flat(tok, 0, np.int32)            # [NC, NBLK, T, 128]
    relv = (i1s & 127).astype(np.float32)
    relg = scatter_flat(relv.astype(BF16), np.float32(999.0), BF16)
    q = np.clip(np.round(bs * QSCALE), 0, NLEV - 1).astype(np.int32)
    qg = scatter_flat(q, 0, np.int32)

    NSB = NBLK // 2
    # merged low/high gather idx per superblock:
    # order = [A tiles 0..2Tq-1 | B tiles 0..2Tq-1] each 128 slots
    tok_sb = tokg.reshape(NC, NSB, 2, T, 128)
    lo = tok_sb[:, :, :, 0:2 * Tq, :].reshape(NC, NSB, 2 * 2 * Tq * 128)
    hi = tok_sb[:, :, :, 2 * Tq:T, :].reshape(NC, NSB, 2 * 2 * Tq * 128)
    i2lo = _wrap16(lo)
    i2hi = _wrap16(hi)
    # bexp LUT idx per superblock: [A tiles 0..T-1 | B tiles 0..T-1]
    q_sb = qg.reshape(NC, NSB, 2 * T * 128)
    qlut = _wrap16(q_sb)

    # S8T one-hot select matrices: [NC, NSB, 128(sites), 2*T*128]
    rel_i = np.full(nblk_tot * cap, 255, dtype=np.int16)
    rel_i[slots] = (i1s & 127).astype(np.int16)
    rel_i = rel_i.reshape(NC, NSB, 1, 2, T * 128)
    s8t = (rel_i == np.arange(128, dtype=np.int16)[None, None, :, None, None])
    s8t = np.ascontiguousarray(s8t.astype(BF16))  # [NC, NSB, 128, 2, T*128]

    rel_blk = relg.reshape(NC, NSB, 2, T, 128).transpose(0, 1, 4, 2, 3)
    rel_blk = np.ascontiguousarray(rel_blk)  # [NC, NSB, 128, 2, T]

    # gaussian LUT: token q -> [G(:, q), G(:, q)] at partition q%128,
    # rank q//128
    centers = np.arange(BOND_EXP, dtype=np.float64) * STEP
    dq = np.arange(NLEV, dtype=np.float64) / QSCALE
    G64 = np.exp(-(((dq[None, :] - centers[:, None]) / STEP) ** 2))  # [64, NLEV]
    Gtok = np.concatenate([G64, G64], axis=0).astype(BF16)  # [128, NLEV]
    # token q lives at partition q%128, rank q//128 (256B = 128 bf16)
    Gtab = np.zeros((128, GRANK, 128), dtype=BF16)
    qidx = np.arange(NLEV)
    Gtab[qidx % 128, qidx // 128, :] = Gtok[:, qidx].T
    Gtab = Gtab.reshape(128, GRANK * 128)

    sites = np.asarray(inputs["sites"], dtype=np.float32)
    sites_pad = np.zeros((cfg.site_pad, SITE_PROPS), dtype=np.float32)
    sites_pad[:n_sites] = sites
    g2s = np.asarray(inputs["graph_to_sites"])
    gid_pad = np.full(cfg.site_pad, 999.0, dtype=np.float32)
    gid_pad[:n_sites] = g2s.astype(np.float32)

    consts = {
        "identity128": np.eye(128, dtype=np.float32),
        "identity128_bf": np.eye(128).astype(BF16),
        "iota512": np.tile(np.arange(cfg.n_graphs, dtype=np.float32),
                           (128, 1)),
        "iota_grp": np.tile(np.tile(np.arange(128, dtype=np.float32), 8),
                            (128, 1)).astype(BF16),
        "ones_col": np.ones((128, 1), dtype=BF16),
        "Wse": np.asarray(inputs["W_se"], dtype=np.float32),
        "bse": np.asarray(inputs["b_se"],
                          dtype=np.float32).reshape(SITE_EMB, 1),
        "Gtab": Gtab,
        "W1": np.asarray(inputs["W1"], dtype=np.float32),
        "b1": np.asarray(inputs["b1"], dtype=np.float32).reshape(H1, 1),
        "W2": np.asarray(inputs["W2"], dtype=np.float32),
        "b2": np.asarray(inputs["b2"], dtype=np.float32).reshape(H2, 1),
        "W3": np.asarray(inputs["W3"], dtype=np.float32),
        "b3": np.asarray(inputs["b3"], dtype=np.float32).reshape(1, 1),
    }
    gate_bias = False
    W_be = np.asarray(inputs["W_be"], dtype=np.float32)
    b_be = np.asarray(inputs["b_be"], dtype=np.float32).reshape(-1)
    for L in (1, 2):
        Wsig = np.asarray(inputs[f"W_sig{L}"], dtype=np.float32)
        Wsof = np.asarray(inputs[f"W_sof{L}"], dtype=np.float32)
        consts[f"W{L}_cat1"] = np.concatenate(
            [Wsig[0:64], Wsof[0:64]], axis=1).astype(BF16)
        s2cat = np.concatenate([Wsig[64:128], Wsof[64:128]], axis=1)
        Z = np.zeros_like(s2cat)
        consts[f"W{L}_s2v0"] = np.concatenate([s2cat, Z], axis=0).astype(BF16)
        consts[f"W{L}_s2v1"] = np.concatenate([Z, s2cat], axis=0).astype(BF16)
        consts[f"W{L}_bb"] = np.concatenate(
            [W_be @ Wsig[128:192], W_be @ Wsof[128:192]], axis=1).astype(BF16)
        bsig = (np.asarray(inputs[f"b_sig{L}"],
                           dtype=np.float32).reshape(-1)
                + b_be @ Wsig[128:192])
        bsof = (np.asarray(inputs[f"b_sof{L}"],
                           dtype=np.float32).reshape(-1)
                + b_be @ Wsof[128:192])
        if np.any(bsig != 0) or np.any(bsof != 0):
            gate_bias = True
        consts[f"b{L}_row2"] = np.tile(
            np.tile(np.concatenate([bsig, bsof]), 8),
            (128, 1)).astype(np.float32)
    cfg.gate_bias = gate_bias
    if not gate_bias:
        for L in (1, 2):
            del consts[f"b{L}_row2"]

    in_maps = []
    for c in range(NC):
        m = dict(consts)
        m["sites_slice"] = sites_pad[c * R:(c + 1) * R]
        m["gid_blk"] = gid_pad[c * R:(c + 1) * R].reshape(NBLK, 128)
        m["i2lo_blk"] = i2lo[c]
        m["i2hi_blk"] = i2hi[c]
        m["qlut_blk"] = qlut[c]
        m["s8t_blk"] = s8t[c]
        m["rel_blk"] = rel_blk[c]
        in_maps.append(m)
    return in_maps


def input_specs(cfg):
    NBLK, T, Tq, R = cfg.nblk, cfg.T, cfg.Tq, cfg.R
    NSB = NBLK // 2
    specs = {
        "sites_slice": ([R, SITE_PROPS], F32),
        "gid_blk": ([NBLK, 128], F32),
        "i2lo_blk": ([NSB, 128, 4 * Tq * 8], I16),
        "i2hi_blk": ([NSB, 128, 4 * Tq * 8], I16),
        "qlut_blk": ([NSB, 128, 2 * T * 8], I16),
        "s8t_blk": ([NSB, 128, 2, T * 128], BF),
        "rel_blk": ([NSB, 128, 2, T], BF),
        "identity128": ([128, 128], F32),
        "identity128_bf": ([128, 128], BF),
        "iota512": ([128, cfg.n_graphs], F32),
        "iota_grp": ([128, 8 * 128], BF),
        "ones_col": ([128, 1], BF),
        "Wse": ([SITE_PROPS, SITE_EMB], F32),
        "bse": ([SITE_EMB, 1], F32),
        "Gtab": ([128, GRANK * 128], BF),
        "W1_cat1": ([SITE_EMB, 2 * SITE_EMB], BF),
        "W2_cat1": ([SITE_EMB, 2 * SITE_EMB], BF),
        "W1_s2v0": ([128, 2 * SITE_EMB], BF),
        "W1_s2v1": ([128, 2 * SITE_EMB], BF),
        "W2_s2v0": ([128, 2 * SITE_EMB], BF),
        "W2_s2v1": ([128, 2 * SITE_EMB], BF),
        "W1_bb": ([SITE_EMB, 2 * SITE_EMB], BF),
        "W2_bb": ([SITE_EMB, 2 * SITE_EMB], BF),
        "W1": ([SITE_EMB, H1], F32), "b1": ([H1, 1], F32),
        "W2": ([H1, H2], F32), "b2": ([H2, 1], F32),
        "W3": ([H2, OUT], F32), "b3": ([1, 1], F32),
    }
    if cfg.gate_bias:
        for L in (1, 2):
            specs[f"b{L}_row2"] = ([128, 8 * 128], F32)
    return specs


def build_bass(cfg):
    nc = bacc.Bacc("TRN2", target_bir_lowering=False, debug=False,
                   num_devices=cfg.n_cores)
    ins = {}
    for name, (shape, dt) in input_specs(cfg).items():
        ins[name] = nc.dram_tensor(name, shape, dt, kind="ExternalInput").ap()
    outs = {
        "out": nc.dram_tensor("out", [cfg.n_graphs, OUT], F32,
                              kind="ExternalOutput").ap()
    }
    with tile.TileContext(nc) as tc:
        build_graph_kernel(nc, tc, ins, outs, cfg)
    nc.compile()
    return nc


_CACHE = {}


def run(inputs, cfg, **kw):
    in_maps = prep_host(inputs, cfg)
    key = (cfg.n_cores, cfg.nblk, cfg.T, cfg.Tq, cfg.site_pad,
           cfg.n_graphs, cfg.gate_bias, cfg.phases, cfg.sbuf_table)
    if key not in _CACHE:
        _CACHE[key] = build_bass(cfg)
    nc = _CACHE[key]
    res = run_bass_kernel_spmd(nc, in_maps, core_ids=list(range(cfg.n_cores)),
                               **kw)
    return res


def kernel(**inputs) -> np.ndarray:
    n_sites = inputs["sites"].shape[0]
    cfg = Cfg(n_cores=8, nblk=98, T=None, n_sites=n_sites)
    res = run(inputs, cfg)
    return np.asarray(res.results[0]["out"], dtype=np.float32)


def build_calib(cfg):
    """Same inputs, trivial program - isolates launch+transfer overhead."""
    nc = bacc.Bacc("TRN2", target_bir_lowering=False, debug=False,
                   num_devices=cfg.n_cores)
    for name, (shape, dt) in input_specs(cfg).items():
        nc.dram_tensor(name, shape, dt, kind="ExternalInput").ap()
    out = nc.dram_tensor("out", [cfg.n_graphs, OUT], F32,
                         kind="ExternalOutput").ap()
    with tile.TileContext(nc) as tc:
        with tc.tile_pool(name="sb", bufs=1) as sb:
            t = sb.tile([1, cfg.n_graphs], F32)
            nc.vector.memset(t[:], 0.0)
            nc.sync.dma_start(out[:].rearrange("g o -> o g"), t[:])
    nc.compile()
    return nc
